# revision 2
# baseline (speedup 1.0000x reference)
"""CascadeTransformerMM Trainium2 kernel.

Problem: B=8, S=512, E=H=2048.
  Wt = ternarize(weight_quant(W))  (host, exact)
  per t:  xq = act_quant(rms_norm(x_t)); f,c,g = acts(xq @ Wt_* + b_*)
          cg = sigmoid(x_t @ W_g.T)
          h  = cg*x + (1-cg)*(f*h_prev + (1-f)*c);  o = g*(f*h_prev + (1-f)*c)

Strategy:
  - Data parallel over batch: core b handles x[b] (512, 2048); no collectives.
  - All matmuls are batched over time in transposed layout:
      Z.T (H,T) = lhsT(=Wt, (E,H)).T @ Xq.T (E,T)
    Activations are quantized to INTEGER levels (xq_int in [-128,127]) and
    stored bf16 => matmul against ternary bf16 weights is numerically EXACT
    (products/partial sums are integers < 2^24, PSUM accumulates fp32).
    The 1/s per-t descale is applied on the PSUM output via a broadcast row.
  - cg matmul uses a hi/lo bf16 split of raw x (x = x_hi + x_lo) => fp32-ish
    precision with two bf16 matmul passes accumulated in the same PSUM.
  - The recurrence h(t) = a(t)*h(t-1) + d(t) with a = (1-cg)*f and
    d = cg*x + (1-cg)*(1-f)*c runs as ONE tensor_tensor_scan per 128-row
    H-tile (state fp32).  o = g * (f*h(t-1) + (1-f)*c).
  - Outputs are transposed back (H,T)->(T,H) on the PE and DMAed out.
"""

import sys

sys.path.insert(0, "/opt/trn_rl_repo")

import numpy as np
import ml_dtypes

import concourse.bass as bass
import concourse.bacc as bacc
import concourse.tile as tile
from concourse import mybir
from concourse.bass import ts
from concourse.bass_utils import run_bass_kernel_spmd
from concourse.masks import make_identity

F32 = mybir.dt.float32
BF16 = mybir.dt.bfloat16

B, S, E, H = 8, 512, 2048, 2048
P = 128
ST = S // P          # 4 S-tiles (natural layout)
KT = E // P          # 16 K-tiles (contraction)
MT = H // P          # 16 M-tiles (output rows)
N_CORES = 8
RC = 12582912.0      # 1.5 * 2**23  (round-to-nearest-even trick)
EPS = 1e-5

AF = mybir.ActivationFunctionType
ALU = mybir.AluOpType


def _host_prep_weights(W):
    """ternarize(weight_quant(W)) in fp32 numpy, exactly as the reference."""
    W = np.asarray(W, dtype=np.float32)
    qmax = np.float32(127.0)
    scale = qmax / (np.float32(np.abs(W).max()) + np.float32(1e-5))
    wq = np.round(np.clip(W * scale, -(qmax + np.float32(1.0)), qmax)) / scale
    sf = np.clip(
        np.float32(1.0) / (np.float32(np.abs(wq).mean()) + np.float32(1e-5)),
        np.float32(1e-4),
        np.float32(1e4),
    )
    return np.sign(wq * sf).astype(np.float32)


def _tile_lhsT(Wm):
    """(E,H) f32 -> (MT, P, KT, P) bf16 slabs; slab[m][p][k][f] = W[k*P+p, m*P+f]."""
    t = Wm.reshape(KT, P, MT, P).transpose(2, 1, 0, 3)
    return np.ascontiguousarray(t).astype(ml_dtypes.bfloat16)


def build_kernel():
    nc = bacc.Bacc("TRN2", target_bir_lowering=False, debug=False,
                   num_devices=N_CORES)

    x_d = nc.declare_dram_parameter("x", (S, E), F32, isOutput=False)
    wf_d = nc.declare_dram_parameter("wf", (MT, P, KT, P), BF16, isOutput=False)
    wc_d = nc.declare_dram_parameter("wc", (MT, P, KT, P), BF16, isOutput=False)
    wg_d = nc.declare_dram_parameter("wg", (MT, P, KT, P), BF16, isOutput=False)
    wgt_d = nc.declare_dram_parameter("wgt", (MT, P, KT, P), BF16, isOutput=False)
    bf_d = nc.declare_dram_parameter("bf", (H,), F32, isOutput=False)
    bc_d = nc.declare_dram_parameter("bc", (H,), F32, isOutput=False)
    bg_d = nc.declare_dram_parameter("bg", (H,), F32, isOutput=False)
    rs_d = nc.declare_dram_parameter("rs", (H,), F32, isOutput=False)
    out_d = nc.declare_dram_parameter("out", (S, H), F32, isOutput=True)

    with tile.TileContext(nc) as tc:
        _emit(nc, tc, x_d, wf_d, wc_d, wg_d, wgt_d, bf_d, bc_d, bg_d, rs_d, out_d)

    nc.compile()
    return nc


def _emit(nc, tc, x_d, wf_d, wc_d, wg_d, wgt_d, bf_d, bc_d, bg_d, rs_d, out_d):
    with tc.tile_pool(name="singles", bufs=1) as singles:
        # ---- persistent constants + transposed activations ----
        id_bf = singles.tile([P, P], BF16)
        make_identity(nc, id_bf)
        id_f32 = singles.tile([P, P], F32)
        make_identity(nc, id_f32)

        bcols = {}
        for name, bd in (("bf", bf_d), ("bc", bc_d), ("bg", bg_d)):
            t = singles.tile([P, MT], F32, tag=f"bcol_{name}")
            nc.sync.dma_start(
                out=t,
                in_=bass.AP(tensor=bd.ap().tensor, offset=0, ap=[[1, P], [P, MT]]),
            )
            bcols[name] = t
        nbf = singles.tile([P, MT], F32)
        nc.vector.tensor_scalar_mul(nbf, bcols["bf"], -1.0)

        xqt = singles.tile([P, KT * S], BF16)   # [p, k*S + t] = xq_int.T
        xht = singles.tile([P, KT * S], BF16)   # x_hi.T
        xlt = singles.tile([P, KT * S], BF16)   # x_lo.T
        sinv_row = singles.tile([1, S], F32)
        sinv_bc = singles.tile([P, S], F32)

        # ================= phase A: x load, rms-norm, quant, transpose ======
        with tc.tile_pool(name="prep_x", bufs=2) as prep_x, \
             tc.tile_pool(name="prep_s", bufs=1) as prep_s, \
             tc.tile_pool(name="prep_n", bufs=2) as prep_n, \
             tc.tile_pool(name="ps_a", bufs=4, space="PSUM") as ps_a:

            scale_bc = prep_s.tile([P, E], F32)
            nc.sync.dma_start(
                out=scale_bc,
                in_=bass.AP(tensor=rs_d.ap().tensor, offset=0, ap=[[0, P], [1, E]]),
            )

            for st in range(ST):
                xt = prep_x.tile([P, E], F32, tag="xt")
                nc.sync.dma_start(out=xt, in_=x_d.ap()[ts(st, P), :])

                xsc = prep_s.tile([P, E], F32, tag="xsc")
                ms = prep_s.tile([P, 1], F32, tag="ms")
                nc.scalar.activation(xsc, xt, AF.Square, accum_out=ms)
                msm = prep_s.tile([P, 1], F32, tag="msm")
                nc.vector.tensor_scalar(msm, ms, 1.0 / E, EPS,
                                        op0=ALU.mult, op1=ALU.add)
                sr = prep_s.tile([P, 1], F32, tag="sr")
                nc.scalar.sqrt(sr, msm)
                rr = prep_s.tile([P, 1], F32, tag="rr")
                nc.vector.reciprocal(rr, sr)

                # xn = (x * rr) * rms_scale
                nc.vector.tensor_scalar_mul(xsc, xt, rr)
                nc.vector.tensor_mul(xsc, xsc, scale_bc)

                am = prep_s.tile([P, 1], F32, tag="am")
                nc.vector.tensor_reduce(am, xsc, axis=mybir.AxisListType.X,
                                        op=ALU.max, apply_absolute_value=True)
                t1 = prep_s.tile([P, 1], F32, tag="t1")
                nc.vector.tensor_scalar_add(t1, am, EPS)
                rec = prep_s.tile([P, 1], F32, tag="rec")
                nc.vector.reciprocal(rec, t1)
                sq = prep_s.tile([P, 1], F32, tag="sq")
                nc.vector.tensor_scalar(sq, rec, 127.0, 1e-3,
                                        op0=ALU.mult, op1=ALU.max)
                nc.vector.tensor_scalar_min(sq, sq, 1e3)
                sinv = prep_s.tile([P, 1], F32, tag="sinv")
                nc.vector.tensor_scalar(sinv, t1, 1.0 / 127.0, 1e-3,
                                        op0=ALU.mult, op1=ALU.max)
                nc.vector.tensor_scalar_min(sinv, sinv, 1e3)

                # quantize in place: xq_int = clip(round(s*xn), -128, 127)
                nc.vector.tensor_scalar(xsc, xsc, sq, RC, op0=ALU.mult, op1=ALU.add)
                nc.vector.tensor_scalar(xsc, xsc, RC, 127.0,
                                        op0=ALU.subtract, op1=ALU.min)
                xq_nat = prep_n.tile([P, E], BF16, tag="xq_nat")
                nc.vector.tensor_scalar_max(xq_nat, xsc, -128.0)

                # hi/lo split of raw x
                xh_nat = prep_n.tile([P, E], BF16, tag="xh_nat")
                nc.vector.tensor_copy(xh_nat, xt)
                xl_nat = prep_n.tile([P, E], BF16, tag="xl_nat")
                nc.vector.tensor_sub(xl_nat, xt, xh_nat)

                # sinv column -> row slice of sinv_row
                pst_s = ps_a.tile([1, P], F32, tag="pst_s")
                nc.tensor.transpose(pst_s, sinv, id_f32)
                nc.scalar.copy(sinv_row[0:1, ts(st, P)], pst_s)

                # transpose the three bf16 tensors into columns of X.T
                for k in range(KT):
                    for src, dst in ((xq_nat, xqt), (xh_nat, xht), (xl_nat, xlt)):
                        pst = ps_a.tile([P, P], BF16, tag="pst")
                        nc.tensor.transpose(pst, src[:, ts(k, P)], id_bf)
                        nc.scalar.copy(
                            dst[:, k * S + st * P: k * S + (st + 1) * P], pst)

        nc.gpsimd.partition_broadcast(sinv_bc, sinv_row)

        # ================= phase B: per-M-tile matmuls + scan + output ======
        with tc.tile_pool(name="wpool", bufs=2) as wpool, \
             tc.tile_pool(name="work", bufs=2) as work, \
             tc.tile_pool(name="zpool", bufs=3) as zpool, \
             tc.tile_pool(name="opool", bufs=2) as opool, \
             tc.tile_pool(name="ps_g", bufs=1, space="PSUM") as ps_g, \
             tc.tile_pool(name="ps_o", bufs=2, space="PSUM") as ps_o:

            for m in range(MT):
                wf_m = wpool.tile([P, KT * P], BF16, tag="wf")
                nc.sync.dma_start(out=wf_m, in_=wf_d.ap()[m])
                wc_m = wpool.tile([P, KT * P], BF16, tag="wc")
                nc.sync.dma_start(out=wc_m, in_=wc_d.ap()[m])
                wg_m = wpool.tile([P, KT * P], BF16, tag="wg")
                nc.sync.dma_start(out=wg_m, in_=wg_d.ap()[m])
                wgt_m = wpool.tile([P, KT * P], BF16, tag="wgt")
                nc.sync.dma_start(out=wgt_m, in_=wgt_d.ap()[m])

                def mm_pass(w_tile, rhs_list, tag):
                    ps = ps_g.tile([P, S], F32, tag=tag)
                    n = len(rhs_list) * KT
                    i = 0
                    for rhs in rhs_list:
                        for k in range(KT):
                            nc.tensor.matmul(
                                ps,
                                lhsT=w_tile[:, ts(k, P)],
                                rhs=rhs[:, k * S: (k + 1) * S],
                                start=(i == 0),
                                stop=(i == n - 1),
                            )
                            i += 1
                    return ps

                # F gate
                ps = mm_pass(wf_m, [xqt], "psF")
                zf = zpool.tile([P, S], F32, tag="z")
                nc.vector.tensor_mul(zf, ps, sinv_bc)
                f_t = work.tile([P, S], BF16, tag="f")
                nc.scalar.activation(f_t, zf, AF.Sigmoid,
                                     bias=bcols["bf"][:, m: m + 1])
                fc_t = work.tile([P, S], BF16, tag="fc")
                nc.scalar.activation(fc_t, zf, AF.Sigmoid, bias=nbf[:, m: m + 1],
                                     scale=-1.0)

                # C gate (silu)
                ps = mm_pass(wc_m, [xqt], "psC")
                zc = zpool.tile([P, S], F32, tag="z")
                nc.vector.tensor_mul(zc, ps, sinv_bc)
                c_t = work.tile([P, S], BF16, tag="c")
                nc.scalar.activation(c_t, zc, AF.Silu,
                                     bias=bcols["bc"][:, m: m + 1])

                # G gate
                ps = mm_pass(wg_m, [xqt], "psG")
                zg = zpool.tile([P, S], F32, tag="z")
                nc.vector.tensor_mul(zg, ps, sinv_bc)
                g_t = work.tile([P, S], BF16, tag="g")
                nc.scalar.activation(g_t, zg, AF.Sigmoid,
                                     bias=bcols["bg"][:, m: m + 1])

                # CG gate: sigmoid(x @ Wg.T), hi + lo accumulated in one PSUM
                ps = mm_pass(wgt_m, [xht, xlt], "psCG")
                cg_t = work.tile([P, S], BF16, tag="cg")
                nc.scalar.activation(cg_t, ps, AF.Sigmoid)
                cgc_t = work.tile([P, S], BF16, tag="cgc")
                nc.scalar.activation(cgc_t, ps, AF.Sigmoid, scale=-1.0)

                # recurrence inputs: a = (1-cg)*f ; d = cg*x + (1-cg)*(1-f)*c
                cw = work.tile([P, S], BF16, tag="cw")      # (1-f)*c
                nc.vector.tensor_mul(cw, fc_t, c_t)
                a_t = work.tile([P, S], BF16, tag="a")
                nc.vector.tensor_mul(a_t, cgc_t, f_t)
                v_t = work.tile([P, S], BF16, tag="v")
                nc.vector.tensor_mul(v_t, cgc_t, cw)
                xf = work.tile([P, S], F32, tag="xf")       # raw x slice (H,T)
                nc.vector.tensor_add(xf, xht[:, m * S: (m + 1) * S],
                                     xlt[:, m * S: (m + 1) * S])
                d_t = work.tile([P, S], F32, tag="d")
                nc.vector.tensor_mul(d_t, cg_t, xf)
                nc.vector.tensor_add(d_t, d_t, v_t)

                hout = opool.tile([P, S], F32, tag="hout")
                nc.vector.tensor_tensor_scan(hout, a_t, d_t, 0.0,
                                             op0=ALU.mult, op1=ALU.add)

                # o = g * (f*h(t-1) + (1-f)*c);  h(-1)=0
                hn = opool.tile([P, S], F32, tag="hn")
                nc.scalar.copy(hn[:, 0:1], cw[:, 0:1])
                nc.vector.tensor_mul(hn[:, 1:S], f_t[:, 1:S], hout[:, 0:S - 1])
                nc.vector.tensor_add(hn[:, 1:S], hn[:, 1:S], cw[:, 1:S])
                nc.vector.tensor_mul(hn, g_t, hn)

                # transpose back (H,T)->(T,H) and store
                for j in range(ST):
                    pso = ps_o.tile([P, P], F32, tag="pso")
                    nc.tensor.transpose(pso, hn[:, ts(j, P)], id_f32)
                    ob = opool.tile([P, P], F32, tag="ob")
                    nc.scalar.copy(ob, pso)
                    nc.sync.dma_start(out=out_d.ap()[ts(j, P), ts(m, P)], in_=ob)


_CACHE = {}


def kernel(x, rms_scale, W_f, W_c, W_g, b_f, b_c, b_g):
    x = np.asarray(x, dtype=np.float32)
    assert x.shape == (B, S, E), x.shape

    if "nc" not in _CACHE:
        _CACHE["nc"] = build_kernel()
    nc = _CACHE["nc"]

    wf = _tile_lhsT(_host_prep_weights(W_f))
    wc = _tile_lhsT(_host_prep_weights(W_c))
    wg = _tile_lhsT(_host_prep_weights(W_g))
    wgt = _tile_lhsT(np.ascontiguousarray(np.asarray(W_g, np.float32).T))

    base = {
        "wf": wf, "wc": wc, "wg": wg, "wgt": wgt,
        "bf": np.asarray(b_f, np.float32),
        "bc": np.asarray(b_c, np.float32),
        "bg": np.asarray(b_g, np.float32),
        "rs": np.asarray(rms_scale, np.float32),
    }
    in_maps = [dict(base, x=np.ascontiguousarray(x[b])) for b in range(B)]

    res = run_bass_kernel_spmd(nc, in_maps, list(range(N_CORES)))
    out = np.stack([res.results[b]["out"] for b in range(B)], axis=0)
    return out.astype(np.float32)
